# revision 53
# baseline (speedup 1.0000x reference)
import ml_dtypes
import numpy as np

import concourse.bass as bass
import concourse.tile as tile
from concourse import bacc, mybir
from concourse.bass_utils import run_bass_kernel_spmd

SL, TL, BS, H = 2048, 2048, 16, 512
NCORES = 8
BPC = BS // NCORES

F32 = mybir.dt.float32
BF16 = mybir.dt.bfloat16
FP8 = mybir.dt.float8e4

NS = SL // 128
NH = H // 128
TCHUNK = 512
NTC = TL // TCHUNK
TPC = TCHUNK // 128
SCALE = 32.0
DSCALE = 16.0
HH = H // 2
HP = HH + 4


def build():
    nc = bacc.Bacc("TRN2", target_bir_lowering=False, debug=False,
                   num_devices=NCORES)
    out_e = nc.dram_tensor("out_e", [SL, BPC, 2 * H], F32,
                           kind="ExternalInput").ap()
    out_d = nc.dram_tensor("out_d", [TL, BPC, H], F32,
                           kind="ExternalInput").ap()
    ident = nc.dram_tensor("ident", [128, 128], BF16,
                           kind="ExternalInput").ap()
    out = nc.dram_tensor("out", [TL, BPC, H], F32,
                         kind="ExternalOutput").ap()

    exp = mybir.ActivationFunctionType.Exp
    dr = mybir.MatmulPerfMode.DoubleRow

    with tile.TileContext(nc) as tc:
        with (
            tc.tile_pool(name="consts", bufs=1) as consts,
            tc.tile_pool(name="stage_e", bufs=6) as stage_e_pool,
            tc.tile_pool(name="stage_d", bufs=4) as stage_d_pool,
            tc.tile_pool(name="oenat", bufs=2 * NS) as oenat_pool,
            tc.tile_pool(name="oet", bufs=2 * NS) as oet_pool,
            tc.tile_pool(name="odt", bufs=2 * NTC) as odt_pool,
            tc.tile_pool(name="pbuf", bufs=8) as p_pool,
            tc.tile_pool(name="d8buf", bufs=2 * NS) as d8_pool,
            tc.tile_pool(name="oe8buf", bufs=2 * NS) as oe8_pool,
            tc.tile_pool(name="osb", bufs=3) as osb_pool,
            tc.tile_pool(name="small", bufs=4) as small_pool,
            tc.tile_pool(name="psS", bufs=3, space="PSUM") as psS_pool,
            tc.tile_pool(name="psC", bufs=3, space="PSUM") as psC_pool,
            tc.tile_pool(name="ptr", bufs=2, space="PSUM") as ptr_pool,
        ):
            ones = consts.tile([128, 1], BF16, tag="ones")
            nc.vector.memset(ones, 1.0)
            onesK1 = consts.tile([1, 128], BF16, tag="onesK1")
            nc.vector.memset(onesK1, 1.0)
            idt = consts.tile([128, 128], BF16, tag="idt")
            nc.sync.dma_start(idt, ident)

            def transpose_tiles(src, dst, eng=None):
                pt = ptr_pool.tile([128, NH * 128], F32, tag="ptr")
                for c in range(NH):
                    nc.tensor.matmul(pt[:, c * 128:(c + 1) * 128],
                                     src[:, c * 128:(c + 1) * 128], idt,
                                     start=True, stop=True)
                if eng is None:
                    nc.vector.tensor_copy(dst, pt)
                else:
                    eng(dst, pt)

            def cast_sc(dst, src):
                nc.scalar.activation(dst, src,
                                     mybir.ActivationFunctionType.Copy)

            class BatchState:
                def __init__(self, b):
                    self.b = b
                    self.oe_tiles = [None] * NS
                    self.oe8_pairs = [None] * (NS // 2)
                    self.oeT_tiles = [None] * NS
                    self.odT_chunks = [None] * NTC
                    self.d8_pairs = {tci: [None] * (NS // 2)
                                     for tci in range(NTC)}
                    self.sd = [None] * NTC
                    self.st = [None] * (NS // 2)
                    self.cs2 = None

            def dma_d(S, ci, split=False):
                if split:
                    S.sd[ci] = [
                        stage_d_pool.tile([128, H], BF16, tag="sd0",
                                          bufs=TPC, name=f"sd0_{k}")
                        for k in range(TPC)]
                    for k in range(TPC):
                        r0 = ci * TCHUNK + k * 128
                        f32k = stage_d_pool.tile([128, H], F32, tag="sdf",
                                                 bufs=2, name=f"sdf_{k}")
                        nc.sync.dma_start(f32k, out_d[r0:r0 + 128, S.b, :])
                        nc.scalar.activation(
                            S.sd[ci][k], f32k,
                            mybir.ActivationFunctionType.Copy)
                    return
                sd = stage_d_pool.tile([128, TPC, H], BF16, tag="sd",
                                       name=f"sd_{S.b}_{ci}")
                S.sd[ci] = sd
                src = out_d[ci * TCHUNK:(ci + 1) * TCHUNK, S.b, :]
                nc.gpsimd.dma_start(
                    sd, src.rearrange("(k p) h -> p k h", p=128))

            def tr_d_split(S, ci):
                odc = odt_pool.tile([128, NH, TCHUNK], FP8, tag="odT",
                                    name=f"odT_{S.b}_{ci}")
                S.odT_chunks[ci] = odc
                for k in range(TPC):
                    transpose_tiles(S.sd[ci][k],
                                    odc[:, :, k * 128:(k + 1) * 128],
                                    eng=cast_sc if k % 2 else None)

            def tr_d(S, ci):
                odc = odt_pool.tile([128, NH, TCHUNK], FP8, tag="odT",
                                    name=f"odT_{S.b}_{ci}")
                S.odT_chunks[ci] = odc
                for k in range(TPC):
                    transpose_tiles(S.sd[ci][:, k, :],
                                    odc[:, :, k * 128:(k + 1) * 128],
                                    eng=cast_sc if k % 2 else None)

            def dma_e(S, j):
                st = stage_e_pool.tile([128, 2, 2 * H], BF16, tag="st",
                                       name=f"st_{S.b}_{j}")
                S.st[j] = st
                src = out_e[j * 256:(j + 1) * 256, S.b, :]
                nc.gpsimd.dma_start(
                    st, src.rearrange("(k p) h -> p k h", p=128))

            def tr_e(S, j):
                st = S.st[j]
                oe8 = oe8_pool.tile([128, 2, 2, HP], FP8, tag="oe8",
                                    name=f"oe8_{S.b}_{j}")
                nc.gpsimd.memset(oe8[:, :, :, HH:HP], 1.0)
                S.oe8_pairs[j] = oe8
                for k in range(2):
                    oe = oenat_pool.tile([128, H], BF16, tag="oe",
                                         name=f"oe_{S.b}_{2 * j + k}")
                    oeT = oet_pool.tile([128, NH, 128], FP8, tag="oeT",
                                        name=f"oeT_{S.b}_{2 * j + k}")
                    S.oe_tiles[2 * j + k] = oe
                    S.oeT_tiles[2 * j + k] = oeT
                    nc.vector.tensor_add(oe, st[:, k, 0:H],
                                         st[:, k, H:2 * H])
                    transpose_tiles(oe, oeT)
                    nc.vector.tensor_copy(oe8[:, k, :, 0:HH], oe)

            def mm1(S, tci, i):
                psS = psS_pool.tile([128, TCHUNK], F32, tag="psS")
                for c2 in range(NH // 2):
                    nc.tensor.matmul(
                        psS,
                        S.oeT_tiles[i][:, 2 * c2:2 * c2 + 2, :],
                        S.odT_chunks[tci][:, 2 * c2:2 * c2 + 2, :],
                        start=(c2 == 0), stop=(c2 == NH // 2 - 1),
                        perf_mode=dr)
                P = p_pool.tile([128, TCHUNK], BF16, tag="P",
                                name=f"P_{S.b}_{tci}_{i}")
                nc.scalar.activation(P, psS, exp,
                                     scale=1.0 / (SCALE * SCALE))
                if i % 2 == 0:
                    d8 = d8_pool.tile([128, 2, TCHUNK], FP8, tag="d8",
                                      name=f"d8_{S.b}_{tci}_{i // 2}")
                    S.d8_pairs[tci][i // 2] = d8
                eng = nc.vector if i % 2 == 0 else nc.gpsimd
                eng.tensor_scalar(S.d8_pairs[tci][i // 2][:, i % 2, :],
                                  P, -1.0, DSCALE,
                                  mybir.AluOpType.add,
                                  mybir.AluOpType.mult)

            def colsum(S):
                pcs = ptr_pool.tile([1, 2, HH], F32, tag="ptr")
                for i in range(NS):
                    nc.tensor.matmul(pcs, ones, S.oe_tiles[i],
                                     start=(i == 0), stop=(i == NS - 1))
                cs2 = small_pool.tile([1, 2, HP], BF16, tag="cs", bufs=2)
                nc.vector.memset(cs2[:, :, HH:HP], float(DSCALE * SL))
                nc.vector.tensor_scalar(cs2[:, :, 0:HH], pcs, DSCALE, None,
                                        mybir.AluOpType.mult)
                S.cs2 = cs2

            def mm2(S, tci, feed=None):
                for tt in range(TPC):
                    if feed is not None:
                        bundle = next(feed, None)
                        if bundle is not None:
                            for op in bundle:
                                op()
                    psA = psC_pool.tile([128, HP], F32, tag="psC")
                    psB = psC_pool.tile([128, HP], F32, tag="psC")
                    nc.tensor.matmul(psA, onesK1, S.cs2[:, 0, :],
                                     start=True, stop=False)
                    nc.tensor.matmul(psB, onesK1, S.cs2[:, 1, :],
                                     start=True, stop=False)
                    for j in range(NS // 2):
                        nc.tensor.matmul(psA,
                                         S.d8_pairs[tci][j][:, :,
                                             tt * 128:(tt + 1) * 128],
                                         S.oe8_pairs[j][:, :, 0, :],
                                         start=False,
                                         stop=(j == NS // 2 - 1),
                                         perf_mode=dr)
                    rc = small_pool.tile([128, 1], F32, tag="rc")
                    nc.vector.reciprocal(rc, psA[:, HH:HH + 1])
                    for j in range(NS // 2):
                        nc.tensor.matmul(psB,
                                         S.d8_pairs[tci][j][:, :,
                                             tt * 128:(tt + 1) * 128],
                                         S.oe8_pairs[j][:, :, 1, :],
                                         start=False,
                                         stop=(j == NS // 2 - 1),
                                         perf_mode=dr)
                    ob = osb_pool.tile([128, H], F32, tag="ob")
                    nc.vector.tensor_scalar(ob[:, 0:HH], psA[:, 0:HH],
                                            rc, None,
                                            mybir.AluOpType.mult)
                    nc.vector.tensor_scalar(ob[:, HH:H], psB[:, 0:HH],
                                            rc, None,
                                            mybir.AluOpType.mult)
                    t0 = tci * TCHUNK + tt * 128
                    nc.sync.dma_start(out[t0:t0 + 128, S.b, :], ob)

            def head_bundles(S, first=False):
                def dD(ci):
                    return lambda: dma_d(S, ci)

                def tD(ci):
                    return lambda: tr_d(S, ci)

                def dE(j):
                    return lambda: dma_e(S, j)

                def tE(j):
                    return lambda: tr_e(S, j)

                def m1(s):
                    return [lambda t=t, s=s: mm1(S, t, s)
                            for t in range(NTC)]

                if first:
                    b0123 = [[lambda: tr_d_split(S, 0)],
                             [tD(1), dE(1), dD(2)],
                             [tE(0), dE(2), dD(3)],
                             [dE(3)]]
                    bundles = b0123 + [
                        [tD(2), dE(4)],
                        [tD(3), dE(5)] + m1(0),
                        [tE(1), dE(6)] + m1(1) + m1(2),
                        [tE(2), dE(7)] + m1(3) + m1(4),
                        [tE(3)] + m1(5) + m1(6),
                        [tE(4)] + m1(7) + m1(8),
                        [tE(5)] + m1(9) + m1(10),
                        [tE(6)] + m1(11) + m1(12),
                        [tE(7)] + m1(13),
                        m1(14) + m1(15),
                        [lambda: colsum(S)],
                    ]
                    return bundles
                return [
                    [tD(0)],
                    [tD(1), dE(1), dD(2)],
                    [tE(0), dE(2), dD(3)],
                    [tE(1), dE(3)],
                    [tD(2), dE(4)],
                    [tD(3), dE(5)] + m1(0),
                    [tE(2), dE(6)] + m1(1) + m1(2),
                    [tE(3), dE(7)] + m1(3) + m1(4),
                    [tE(4)] + m1(5) + m1(6),
                    [tE(5)] + m1(7) + m1(8),
                    [tE(6)] + m1(9) + m1(10),
                    [tE(7)] + m1(11) + m1(12),
                    m1(13),
                    m1(14),
                    m1(15),
                    [lambda: colsum(S)],
                ]

            S0 = BatchState(0)
            dma_d(S0, 0, split=True)
            dma_d(S0, 1)
            dma_e(S0, 0)

            warm = consts.tile([128, TCHUNK], BF16, tag="warm")
            nc.vector.memset(warm, 0.25)
            wt = ptr_pool.tile([128, TCHUNK], F32, tag="ptr")
            for _ in range(28):
                nc.tensor.matmul(wt, warm[:, 0:128], warm,
                                 start=True, stop=True)

            S1 = BatchState(1)
            for bundle in head_bundles(S0, first=True):
                for op in bundle:
                    op()
            dma_d(S1, 0)
            dma_d(S1, 1)
            dma_e(S1, 0)
            feed = iter(head_bundles(S1))
            for tci in range(NTC):
                mm2(S0, tci, feed=feed)
            for bundle in feed:
                for op in bundle:
                    op()
            for tci in range(NTC):
                mm2(S1, tci)

    nc.compile()
    return nc


_nc = None
last_result = None
_IDENT = (np.eye(128) * SCALE).astype(ml_dtypes.bfloat16)


def kernel(in_e=None, out_e=None, out_d=None, _trace=False, **_unused):
    global _nc, last_result
    if _nc is None:
        _nc = build()
    out_e = np.asarray(out_e, dtype=np.float32)
    out_d = np.asarray(out_d, dtype=np.float32)
    in_maps = []
    for c in range(NCORES):
        sl = slice(c * BPC, (c + 1) * BPC)
        in_maps.append({
            "out_e": np.ascontiguousarray(out_e[:, sl, :]),
            "out_d": np.ascontiguousarray(out_d[:, sl, :]),
            "ident": _IDENT,
        })
    last_result = run_bass_kernel_spmd(_nc, in_maps,
                                       core_ids=list(range(NCORES)),
                                       trace=_trace)
    return np.concatenate(
        [np.asarray(last_result.results[c]["out"]) for c in range(NCORES)],
        axis=1).astype(np.float32)



# revision 54
# speedup vs baseline: 1.0043x; 1.0043x over previous
import ml_dtypes
import numpy as np

import concourse.bass as bass
import concourse.tile as tile
from concourse import bacc, mybir
from concourse.bass_utils import run_bass_kernel_spmd

SL, TL, BS, H = 2048, 2048, 16, 512
NCORES = 8
BPC = BS // NCORES

F32 = mybir.dt.float32
BF16 = mybir.dt.bfloat16
FP8 = mybir.dt.float8e4

NS = SL // 128
NH = H // 128
TCHUNK = 512
NTC = TL // TCHUNK
TPC = TCHUNK // 128
SCALE = 32.0
DSCALE = 16.0
HH = H // 2
HP = HH + 4


def build():
    nc = bacc.Bacc("TRN2", target_bir_lowering=False, debug=False,
                   num_devices=NCORES)
    out_e = nc.dram_tensor("out_e", [SL, BPC, 2 * H], F32,
                           kind="ExternalInput").ap()
    out_d = nc.dram_tensor("out_d", [TL, BPC, H], F32,
                           kind="ExternalInput").ap()
    ident = nc.dram_tensor("ident", [128, 128], BF16,
                           kind="ExternalInput").ap()
    out = nc.dram_tensor("out", [TL, BPC, H], F32,
                         kind="ExternalOutput").ap()

    exp = mybir.ActivationFunctionType.Exp
    dr = mybir.MatmulPerfMode.DoubleRow

    with tile.TileContext(nc) as tc:
        with (
            tc.tile_pool(name="consts", bufs=1) as consts,
            tc.tile_pool(name="stage_e", bufs=6) as stage_e_pool,
            tc.tile_pool(name="stage_d", bufs=4) as stage_d_pool,
            tc.tile_pool(name="oenat", bufs=2 * NS) as oenat_pool,
            tc.tile_pool(name="oet", bufs=2 * NS) as oet_pool,
            tc.tile_pool(name="odt", bufs=2 * NTC) as odt_pool,
            tc.tile_pool(name="pbuf", bufs=8) as p_pool,
            tc.tile_pool(name="d8buf", bufs=2 * NS) as d8_pool,
            tc.tile_pool(name="oe8buf", bufs=2 * NS) as oe8_pool,
            tc.tile_pool(name="osb", bufs=3) as osb_pool,
            tc.tile_pool(name="small", bufs=4) as small_pool,
            tc.tile_pool(name="psS", bufs=3, space="PSUM") as psS_pool,
            tc.tile_pool(name="psC", bufs=3, space="PSUM") as psC_pool,
            tc.tile_pool(name="ptr", bufs=2, space="PSUM") as ptr_pool,
        ):
            ones = consts.tile([128, 1], BF16, tag="ones")
            nc.vector.memset(ones, 1.0)
            onesK1 = consts.tile([1, 128], BF16, tag="onesK1")
            nc.vector.memset(onesK1, 1.0)
            idt = consts.tile([128, 128], BF16, tag="idt")
            nc.sync.dma_start(idt, ident)

            def transpose_tiles(src, dst, eng=None):
                pt = ptr_pool.tile([128, NH * 128], F32, tag="ptr")
                for c in range(NH):
                    nc.tensor.matmul(pt[:, c * 128:(c + 1) * 128],
                                     src[:, c * 128:(c + 1) * 128], idt,
                                     start=True, stop=True)
                if eng is None:
                    nc.vector.tensor_copy(dst, pt)
                else:
                    eng(dst, pt)

            def cast_sc(dst, src):
                nc.scalar.activation(dst, src,
                                     mybir.ActivationFunctionType.Copy)

            class BatchState:
                def __init__(self, b):
                    self.b = b
                    self.oe_tiles = [None] * NS
                    self.oe8_pairs = [None] * (NS // 2)
                    self.oeT_tiles = [None] * NS
                    self.odT_chunks = [None] * NTC
                    self.d8_pairs = {tci: [None] * (NS // 2)
                                     for tci in range(NTC)}
                    self.sd = [None] * NTC
                    self.st = [None] * (NS // 2)
                    self.cs2 = None

            def dma_d(S, ci, split=False):
                if split:
                    S.sd[ci] = [
                        stage_d_pool.tile([128, H], BF16, tag="sd0",
                                          bufs=TPC, name=f"sd0_{k}")
                        for k in range(TPC)]
                    for k in range(TPC):
                        r0 = ci * TCHUNK + k * 128
                        f32k = stage_d_pool.tile([128, H], F32, tag="sdf",
                                                 bufs=2, name=f"sdf_{k}")
                        nc.sync.dma_start(f32k, out_d[r0:r0 + 128, S.b, :])
                        nc.scalar.activation(
                            S.sd[ci][k], f32k,
                            mybir.ActivationFunctionType.Copy)
                    return
                sd = stage_d_pool.tile([128, TPC, H], BF16, tag="sd",
                                       name=f"sd_{S.b}_{ci}")
                S.sd[ci] = sd
                src = out_d[ci * TCHUNK:(ci + 1) * TCHUNK, S.b, :]
                nc.gpsimd.dma_start(
                    sd, src.rearrange("(k p) h -> p k h", p=128))

            def tr_d_split(S, ci):
                odc = odt_pool.tile([128, NH, TCHUNK], FP8, tag="odT",
                                    name=f"odT_{S.b}_{ci}")
                S.odT_chunks[ci] = odc
                for k in range(TPC):
                    transpose_tiles(S.sd[ci][k],
                                    odc[:, :, k * 128:(k + 1) * 128],
                                    eng=cast_sc if k % 2 else None)

            def tr_d(S, ci):
                odc = odt_pool.tile([128, NH, TCHUNK], FP8, tag="odT",
                                    name=f"odT_{S.b}_{ci}")
                S.odT_chunks[ci] = odc
                for k in range(TPC):
                    transpose_tiles(S.sd[ci][:, k, :],
                                    odc[:, :, k * 128:(k + 1) * 128],
                                    eng=cast_sc if k % 2 else None)

            def dma_e(S, j):
                st = stage_e_pool.tile([128, 2, 2 * H], BF16, tag="st",
                                       name=f"st_{S.b}_{j}")
                S.st[j] = st
                src = out_e[j * 256:(j + 1) * 256, S.b, :]
                nc.gpsimd.dma_start(
                    st, src.rearrange("(k p) h -> p k h", p=128))

            def tr_e(S, j):
                st = S.st[j]
                oe8 = oe8_pool.tile([128, 2, 2, HP], FP8, tag="oe8",
                                    name=f"oe8_{S.b}_{j}")
                nc.gpsimd.memset(oe8[:, :, :, HH:HP], 1.0)
                S.oe8_pairs[j] = oe8
                for k in range(2):
                    oe = oenat_pool.tile([128, H], BF16, tag="oe",
                                         name=f"oe_{S.b}_{2 * j + k}")
                    oeT = oet_pool.tile([128, NH, 128], FP8, tag="oeT",
                                        name=f"oeT_{S.b}_{2 * j + k}")
                    S.oe_tiles[2 * j + k] = oe
                    S.oeT_tiles[2 * j + k] = oeT
                    nc.vector.tensor_add(oe, st[:, k, 0:H],
                                         st[:, k, H:2 * H])
                    transpose_tiles(oe, oeT)
                    nc.vector.tensor_copy(oe8[:, k, :, 0:HH], oe)

            def mm1(S, tci, i):
                psS = psS_pool.tile([128, TCHUNK], F32, tag="psS")
                for c2 in range(NH // 2):
                    nc.tensor.matmul(
                        psS,
                        S.oeT_tiles[i][:, 2 * c2:2 * c2 + 2, :],
                        S.odT_chunks[tci][:, 2 * c2:2 * c2 + 2, :],
                        start=(c2 == 0), stop=(c2 == NH // 2 - 1),
                        perf_mode=dr)
                P = p_pool.tile([128, TCHUNK], BF16, tag="P",
                                name=f"P_{S.b}_{tci}_{i}")
                nc.scalar.activation(P, psS, exp,
                                     scale=1.0 / (SCALE * SCALE))
                if i % 2 == 0:
                    d8 = d8_pool.tile([128, 2, TCHUNK], FP8, tag="d8",
                                      name=f"d8_{S.b}_{tci}_{i // 2}")
                    S.d8_pairs[tci][i // 2] = d8
                eng = nc.vector if i % 2 == 0 else nc.gpsimd
                eng.tensor_scalar(S.d8_pairs[tci][i // 2][:, i % 2, :],
                                  P, -1.0, DSCALE,
                                  mybir.AluOpType.add,
                                  mybir.AluOpType.mult)

            def colsum(S):
                pcs = ptr_pool.tile([1, 2, HH], F32, tag="ptr")
                for i in range(NS):
                    nc.tensor.matmul(pcs, ones, S.oe_tiles[i],
                                     start=(i == 0), stop=(i == NS - 1))
                cs2 = small_pool.tile([1, 2, HP], BF16, tag="cs", bufs=2)
                nc.vector.memset(cs2[:, :, HH:HP], float(DSCALE * SL))
                nc.vector.tensor_scalar(cs2[:, :, 0:HH], pcs, DSCALE, None,
                                        mybir.AluOpType.mult)
                S.cs2 = cs2

            def mm2(S, tci, feed=None):
                for tt in range(TPC):
                    if feed is not None:
                        bundle = next(feed, None)
                        if bundle is not None:
                            for op in bundle:
                                op()
                    psA = psC_pool.tile([128, HP], F32, tag="psC")
                    psB = psC_pool.tile([128, HP], F32, tag="psC")
                    nc.tensor.matmul(psA, onesK1, S.cs2[:, 0, :],
                                     start=True, stop=False)
                    nc.tensor.matmul(psB, onesK1, S.cs2[:, 1, :],
                                     start=True, stop=False)
                    for j in range(NS // 2):
                        nc.tensor.matmul(psA,
                                         S.d8_pairs[tci][j][:, :,
                                             tt * 128:(tt + 1) * 128],
                                         S.oe8_pairs[j][:, :, 0, :],
                                         start=False,
                                         stop=(j == NS // 2 - 1),
                                         perf_mode=dr)
                    rc = small_pool.tile([128, 1], F32, tag="rc")
                    nc.vector.reciprocal(rc, psA[:, HH:HH + 1])
                    for j in range(NS // 2):
                        nc.tensor.matmul(psB,
                                         S.d8_pairs[tci][j][:, :,
                                             tt * 128:(tt + 1) * 128],
                                         S.oe8_pairs[j][:, :, 1, :],
                                         start=False,
                                         stop=(j == NS // 2 - 1),
                                         perf_mode=dr)
                    ob = osb_pool.tile([128, H], F32, tag="ob")
                    nc.vector.tensor_scalar(ob[:, 0:HH], psA[:, 0:HH],
                                            rc, None,
                                            mybir.AluOpType.mult)
                    nc.vector.tensor_scalar(ob[:, HH:H], psB[:, 0:HH],
                                            rc, None,
                                            mybir.AluOpType.mult)
                    t0 = tci * TCHUNK + tt * 128
                    nc.sync.dma_start(out[t0:t0 + 128, S.b, :], ob)

            def head_bundles(S, first=False):
                def dD(ci):
                    return lambda: dma_d(S, ci)

                def tD(ci):
                    return lambda: tr_d(S, ci)

                def dE(j):
                    return lambda: dma_e(S, j)

                def tE(j):
                    return lambda: tr_e(S, j)

                def m1(s):
                    return [lambda t=t, s=s: mm1(S, t, s)
                            for t in range(NTC)]

                if first:
                    b0123 = [[lambda: tr_d_split(S, 0)],
                             [tD(1), dE(1), dD(2)],
                             [tE(0), dE(2), dD(3)],
                             [dE(3)]]
                    bundles = b0123 + [
                        [tD(2), dE(4)],
                        [tD(3), dE(5)] + m1(0),
                        [tE(1), dE(6)] + m1(1) + m1(2),
                        [tE(2), dE(7)] + m1(3) + m1(4),
                        [tE(3)] + m1(5) + m1(6),
                        [tE(4)] + m1(7) + m1(8),
                        [tE(5)] + m1(9) + m1(10),
                        [tE(6)] + m1(11) + m1(12),
                        [tE(7)] + m1(13),
                        m1(14) + m1(15),
                        [lambda: colsum(S)],
                    ]
                    return bundles
                return [
                    [dD(0), dD(1)], [dE(0)],
                    [tD(0), dE(1), dD(2)],
                    [tD(1), dE(2), dD(3)],
                    [tE(0), dE(3)],
                    [tD(2), dE(4)],
                    [tD(3), dE(5)] + m1(0),
                    [tE(1), dE(6)] + m1(1) + m1(2),
                    [tE(2), dE(7)] + m1(3) + m1(4),
                    [tE(3)] + m1(5) + m1(6),
                    [tE(4)] + m1(7) + m1(8),
                    [tE(5)] + m1(9) + m1(10),
                    [tE(6)] + m1(11) + m1(12),
                    [tE(7)] + m1(13),
                    m1(14) + m1(15),
                    [lambda: colsum(S)],
                ]

            S0 = BatchState(0)
            dma_d(S0, 0, split=True)
            dma_d(S0, 1)
            dma_e(S0, 0)

            warm = consts.tile([128, TCHUNK], BF16, tag="warm")
            nc.vector.memset(warm, 0.25)
            wt = ptr_pool.tile([128, TCHUNK], F32, tag="ptr")
            for _ in range(28):
                nc.tensor.matmul(wt, warm[:, 0:128], warm,
                                 start=True, stop=True)

            S1 = BatchState(1)
            for bundle in head_bundles(S0, first=True):
                for op in bundle:
                    op()
            feed = iter(head_bundles(S1))
            for tci in range(NTC):
                mm2(S0, tci, feed=feed)
            for bundle in feed:
                for op in bundle:
                    op()
            for tci in range(NTC):
                mm2(S1, tci)

    nc.compile()
    return nc


_nc = None
last_result = None
_IDENT = (np.eye(128) * SCALE).astype(ml_dtypes.bfloat16)


def kernel(in_e=None, out_e=None, out_d=None, _trace=False, **_unused):
    global _nc, last_result
    if _nc is None:
        _nc = build()
    out_e = np.asarray(out_e, dtype=np.float32)
    out_d = np.asarray(out_d, dtype=np.float32)
    in_maps = []
    for c in range(NCORES):
        sl = slice(c * BPC, (c + 1) * BPC)
        in_maps.append({
            "out_e": np.ascontiguousarray(out_e[:, sl, :]),
            "out_d": np.ascontiguousarray(out_d[:, sl, :]),
            "ident": _IDENT,
        })
    last_result = run_bass_kernel_spmd(_nc, in_maps,
                                       core_ids=list(range(NCORES)),
                                       trace=_trace)
    return np.concatenate(
        [np.asarray(last_result.results[c]["out"]) for c in range(NCORES)],
        axis=1).astype(np.float32)

